# revision 29
# baseline (speedup 1.0000x reference)
"""Trainium2 Bass kernel for an XNOR-Net BasicBlock (dense_cnn).

Computes, for x [64,256,56,56] (NCHW):
    h = xnor_conv3x3(x, w1) -> bn1 -> hardtanh -> xnor_conv3x3 -> bn2
    out = relu(h + x)

where xnor_conv binarizes activations with sign() and weights with
sign()*mean(|w|) (per output channel).

Strategy (v10, fp8 DoubleRow at ~157 TF/s/core; the stream of 2016
matmuls x 189ns is the roofline, so scheduling focuses on prologue,
epilogue, and keeping the PE p-state hot):
  - Data-parallel over batch: 8 images per NeuronCore x 8 cores.
  - Binarized activations (+-1) are exact in fp8e4; conv = 9 shifted
    matmuls per 3x3 tap with fp32 PSUM accumulation (exact integers).
  - perf_mode=DoubleRow contracts K=256 (both 128-channel blocks) per
    matmul: lhsT [128,2,128], rhs [128,2,448]. DoubleRow requires a 3D
    rhs AP with contiguous N, so sign planes are stored 3x, one copy per
    kj column shift, with row stride 56 (58 rows x 56 cols, borders 0).
    Window for tap (ki,kj), out-row-chunk r0 is then the contiguous run
    plane[kj][:, :, (r0+ki)*W : +N].
  - Chunks are processed in pairs sharing one 2-bank PSUM tile [128,896]
    (each matmul still targets a single bank), halving evacuation ops.
  - Epilogue fusions: conv1 evac = Sign(a1*psum + c1) on ScalarE writing
    the kj=1 plane (kj=0 copy on DVE, kj=2 on GpSimd); conv2 evac =
    DVE (psum*a2)+x' then DVE max(.,0), where x' = x + c2 is prebiased
    once per image on ScalarE. All per-channel constants (alpha, bn
    scale/bias) are folded on the host. hardtanh is a no-op for the
    final output because conv2 only consumes sign(h).
  - Prologue: N_WARM dummy matmuls hold the PE at full clock while
    image 0 streams in on a banded gpsimd DMA queue (weights ride the
    sync engine's queue); image 0 runs a hand-interleaved wave schedule
    (conv2 groups slot in once their conv1 halo exists) because all 8
    cores burst-fetch at t=0 and DMA latency dominates.
  - Steady state: image i+1's binarize/prebias are emitted between
    conv1(i) and conv2(i) so every engine queue drains before conv1 of
    the next image; the last image streams each finished conv2 group to
    DRAM from ScalarE and splits the final chunk's evacuation in half.

Layouts (per core):
  x DRAM     [8, 2, 128, 3136]   (img, c_blk, c_in_blk, h*w) fp32
  w DRAM     [2, 128, 9, 2, 128] (co_blk, ci, tap, ci_blk, co) fp8 sign
  cn DRAM    [2, 128, 4]         (co_blk, co, {a1,c1,a2,c2}) fp32
  out DRAM   [8, 2, 128, 3136]   (img, co_blk, co, h*w) fp32
"""

import os
import numpy as np

N, C, H, W = 64, 256, 56, 56
EPS = 1e-5
N_CORES = 8
IMG_PER_CORE = N // N_CORES
A = 2                     # channel blocks of 128
ROWS = H + 2              # padded rows in a plane
PLANE = ROWS * W          # 3248 (multiple of 16 for DoubleRow dim1 step)
RCH = 8                   # output rows per PSUM chunk
CHUNK = RCH * W           # 448 fp32 <= 512 (one PSUM bank)
HW = H * W
GROUPS = [(0, 1), (2, 3), (4, 5), (6,)]   # chunk pairs -> one PSUM tile
GROUPS0 = [(0,), (1,), (2, 3), (4, 5), (6,)]  # img-0 conv1: singles first
TAPS = [1, 4, 7, 0, 3, 6, 2, 5, 8]        # kj=1 taps first (plane-prep overlap)
BANDS0 = ((0, 9), (9, 24), (24, 40), (40, H))  # image-0 row bands
N_WARM = 24                               # PE p-state warm-up dummy matmuls

_CACHE = {}
LAST_RESULT = None


def _build_program(n_img):
    import concourse.bacc as bacc
    import concourse.mybir as mybir
    import concourse.tile as tile

    dt = mybir.dt
    AF = mybir.ActivationFunctionType
    OP = mybir.AluOpType
    DR = mybir.MatmulPerfMode.DoubleRow

    nc = bacc.Bacc("TRN2", target_bir_lowering=False, debug=False)

    x_d = nc.dram_tensor("x", [n_img, A, 128, HW], dt.float32, kind="ExternalInput")
    w1_d = nc.dram_tensor("w1t", [A, 128, 9, A, 128], dt.float8e4, kind="ExternalInput")
    w2_d = nc.dram_tensor("w2t", [A, 128, 9, A, 128], dt.float8e4, kind="ExternalInput")
    cn_d = nc.dram_tensor("cn", [A, 128, 4], dt.float32, kind="ExternalInput")
    out_d = nc.dram_tensor("out", [n_img, A, 128, HW], dt.float32, kind="ExternalOutput")

    with tile.TileContext(nc) as tc:
        with (
            tc.tile_pool(name="consts", bufs=1) as consts,
            tc.tile_pool(name="planes", bufs=1) as planes,
            tc.tile_pool(name="xin", bufs=2) as xin,
            tc.tile_pool(name="outp", bufs=1) as outp,
            tc.tile_pool(name="evac", bufs=3) as evac,
            tc.tile_pool(name="psum", bufs=1, space="PSUM") as psum,
        ):
            # image-0 input DMA in row bands (a-interleaved) on the gpsimd
            # queue so binarization starts as soon as the first band lands;
            # all weights/consts go down the idle sync engine's queue in
            # urgency order (w1 feeds conv1(0), w2 only from ~20us)
            x_tiles = {}
            x0 = xin.tile([128, A, HW], dt.float32, tag="x_t", name="x_0")

            ws = {}
            cns = []
            ws[(0, 0)] = consts.tile([128, 9, A, 128], dt.float8e4, tag="w0_0",
                                     name="w0_0")
            nc.sync.dma_start(out=ws[(0, 0)][:], in_=w1_d[0])
            for b in range(A):
                t = consts.tile([128, 4], dt.float32, tag=f"cn_{b}", name=f"cn_{b}")
                nc.sync.dma_start(out=t[:], in_=cn_d[b])
                cns.append(t)
            for conv, b, w_d in ((0, 1, w1_d), (1, 0, w2_d), (1, 1, w2_d)):
                t = consts.tile([128, 9, A, 128], dt.float8e4, tag=f"w{conv}_{b}",
                                name=f"w{conv}_{b}")
                nc.sync.dma_start(out=t[:], in_=w_d[b])
                ws[(conv, b)] = t

            for lo, hi in BANDS0:
                for a in range(A):
                    nc.gpsimd.dma_start(out=x0[:, a, lo * W:hi * W],
                                        in_=x_d[0, a][:, lo * W:hi * W])
            x_tiles[0] = x0

            # PE p-state warm-up: dummy matmuls on a zeroed tile keep the
            # tensor engine busy from ~7.6us so the real stream starts at
            # full clock instead of ramping through the 1.2GHz p-state
            warm = consts.tile([128, 2, CHUNK], dt.float8e4, tag="warm", name="warm")
            nc.vector.memset(warm[:], 0.0)
            wps = psum.tile([128, 512], dt.float32, tag="ps1", bufs=2, name="warm_ps")
            for _ in range(N_WARM):
                nc.tensor.matmul(wps[:, 0:CHUNK], lhsT=warm[:, :, 0:128],
                                 rhs=warm[:], start=True, stop=True,
                                 perf_mode=DR)

            # sign planes [128, kj, c_blk, 58 rows, 56 cols] fp8, borders 0,
            # ping-ponged across images. plane[kj][.., rr, j] = xpad[.., rr, j+kj]
            bxp = [planes.tile([128, 3, A, ROWS, W], dt.float8e4, tag=f"bxp{j}",
                               name=f"bxp{j}") for j in range(2)]
            s2p = [planes.tile([128, 3, A, ROWS, W], dt.float8e4, tag=f"s2p{j}",
                               name=f"s2p{j}") for j in range(2)]
            for t in (*bxp, *s2p):
                # border-only init: zero rows 0/57 (all kj) and the padding
                # columns never overwritten per image (kj0 col 0, kj2 col W-1)
                nc.vector.memset(t[:, :, :, 0, :], 0.0)
                nc.vector.memset(t[:, :, :, ROWS - 1, :], 0.0)
                nc.vector.memset(t[:, 0, :, :, 0:1], 0.0)
                nc.vector.memset(t[:, 2, :, :, W - 1:W], 0.0)

            BANK = 512

            def conv_group(src, conv, b, group, ps):
                flat = src.rearrange("p kj a r c -> p kj a (r c)")
                for n_, t_ in enumerate(TAPS):
                    ki, kj = divmod(t_, 3)
                    for gi, ch in enumerate(group):
                        r0 = ch * RCH
                        nc.tensor.matmul(
                            ps[:, gi * BANK:gi * BANK + CHUNK],
                            lhsT=ws[(conv, b)][:, t_, :, :],
                            rhs=flat[:, kj, :, (r0 + ki) * W:(r0 + ki) * W + CHUNK],
                            start=(n_ == 0), stop=(n_ == 8),
                            perf_mode=DR,
                        )

            def psum_tile(group, nm):
                # chunks live at bank-aligned offsets; tail 64 fp32/bank unused
                return psum.tile([128, len(group) * BANK], dt.float32,
                                 tag=f"ps{len(group)}", bufs=3 if len(group) > 1 else 2,
                                 name=nm)

            def psum_chunks(ps, group):
                # [128, G, 448] view of the used part of each bank
                return ps.rearrange("p (g x) -> p g x", x=BANK)[:, :, 0:CHUNK]

            def fetch(i):
                if i not in x_tiles:
                    x_t = xin.tile([128, A, HW], dt.float32, tag="x_t", name=f"x_{i}")
                    nc.gpsimd.dma_start(out=x_t[:],
                                        in_=x_d[i].rearrange("a k s -> k a s"))
                    x_tiles[i] = x_t
                return x_tiles[i]

            def band_acts(i, lo, hi):
                # binarize rows [lo,hi): kj=1 sign on ScalarE, kj=0/2 as DVE
                # shifted copies, split per c_blk so the a=0 copies overlap
                # the a=1 sign pass
                j = i % 2
                xv = x_tiles[i].rearrange("p a (r c) -> p a r c", c=W)
                for a in range(A):
                    nc.scalar.activation(
                        out=bxp[j][:, 1, a, 1 + lo:1 + hi, :],
                        in_=xv[:, a, lo:hi, :],
                        func=AF.Sign,
                    )
                for a in range(A):
                    nc.vector.tensor_copy(
                        out=bxp[j][:, 0, a, 1 + lo:1 + hi, 1:W],
                        in_=bxp[j][:, 1, a, 1 + lo:1 + hi, 0:W - 1])
                    nc.vector.tensor_copy(
                        out=bxp[j][:, 2, a, 1 + lo:1 + hi, 0:W - 1],
                        in_=bxp[j][:, 1, a, 1 + lo:1 + hi, 1:W])

            def conv1_group(i, b, group):
                # conv1 -> fused bn1+sign -> s2p (x3 shifted)
                j = i % 2
                gr = len(group) * RCH
                r0 = group[0] * RCH
                ps = psum_tile(group, f"ps1_{i}_{b}_{group[0]}")
                conv_group(bxp[j], 0, b, group, ps)
                nc.scalar.activation(
                    out=s2p[j][:, 1, b, 1 + r0:1 + r0 + gr, :],
                    in_=psum_chunks(ps, group).rearrange(
                        "p g (r c) -> p g r c", c=W),
                    func=AF.Sign,
                    bias=cns[b][:, 1:2],
                    scale=cns[b][:, 0:1],
                )
                nc.vector.tensor_copy(
                    out=s2p[j][:, 0, b, 1 + r0:1 + r0 + gr, 1:W],
                    in_=s2p[j][:, 1, b, 1 + r0:1 + r0 + gr, 0:W - 1])
                nc.vector.tensor_copy(
                    out=s2p[j][:, 2, b, 1 + r0:1 + r0 + gr, 0:W - 1],
                    in_=s2p[j][:, 1, b, 1 + r0:1 + r0 + gr, 1:W])

            out_tiles = {}

            def get_out(i):
                if i not in out_tiles:
                    out_tiles[i] = outp.tile([128, A, HW], dt.float32,
                                             tag="out_t", name=f"out_{i}")
                return out_tiles[i]

            def conv2_group(i, b, group):
                # conv2 -> DVE (psum*a2)+x' -> DVE max(.,0); the last image
                # streams each finished group to DRAM (ScalarE-issued), and
                # its very last chunk evacuates in two halves for a tighter
                # tail after the final matmul
                j = i % 2
                last = i == n_img - 1
                out_t = get_out(i)
                x_t = x_tiles[i]
                gn = len(group) * CHUNK
                s0 = group[0] * CHUNK
                ps = psum_tile(group, f"ps2_{i}_{b}_{group[0]}")
                conv_group(s2p[j], 1, b, group, ps)
                rr = evac.tile([128, 2 * CHUNK], dt.float32, tag="rr",
                               name=f"rr_{i}_{b}_{group[0]}")
                final = last and b == A - 1 and group is GROUPS[-1]
                pv = psum_chunks(ps, group)
                for pp in range(2 if final else 1):
                    on = gn // 2 if final else gn
                    ro = pp * on
                    o0 = s0 + ro
                    nc.vector.scalar_tensor_tensor(
                        out=rr[:, ro:ro + on],
                        in0=pv[:, :, ro:ro + on] if final else pv,
                        scalar=cns[b][:, 2:3],
                        in1=x_t[:, b, o0:o0 + on],
                        op0=OP.mult, op1=OP.add)
                    nc.vector.tensor_scalar_max(
                        out=out_t[:, b, o0:o0 + on],
                        in0=rr[:, ro:ro + on],
                        scalar1=0.0)
                    if last:
                        nc.scalar.dma_start(
                            out=out_d[i, b][:, o0:o0 + on],
                            in_=out_t[:, b, o0:o0 + on])

            # ---- image 0: hand-interleaved wave schedule. The prologue is
            # DMA-latency-bound (all 8 cores burst-fetch at once), so conv2
            # groups interleave with conv1 as soon as their halo (conv1
            # chunks 2k-1..2k+2) exists, giving the PE extra work per input
            # byte while the bands stream in. ScalarE order: bands early,
            # evacs/prebias in the gaps.
            fetch(0)
            band_acts(0, *BANDS0[0])
            band_acts(0, *BANDS0[1])
            for b in range(A):
                conv1_group(0, b, GROUPS0[0])
            band_acts(0, *BANDS0[2])
            for b in range(A):
                conv1_group(0, b, GROUPS0[1])
            band_acts(0, *BANDS0[3])
            for b in range(A):
                conv1_group(0, b, GROUPS0[2])
            # prebias (x' = x + c2) MUST precede the first conv2 group's
            # evacuation in program order (its stt reads x'); ScalarE runs
            # it in the gap while the PE does conv2(0,1) matmuls
            for b in range(A):
                nc.scalar.add(out=x0[:, b, :], in_=x0[:, b, :],
                              add=cns[b][:, 3:4])
            for b in range(A):
                conv1_group(0, b, GROUPS0[3])
            for b in range(A):
                conv1_group(0, b, GROUPS0[4])
            for b in range(A):
                conv2_group(0, b, GROUPS[0])
            for b in range(A):
                conv2_group(0, b, GROUPS[1])
            for b in range(A):
                conv2_group(0, b, GROUPS[2])

            # prep image 1 while image 0's tail computes
            fetch(1)
            band_acts(1, 0, H)
            for b in range(A):
                nc.scalar.add(out=x_tiles[1][:, b, :], in_=x_tiles[1][:, b, :],
                              add=cns[b][:, 3:4])
            for b in range(A):
                conv2_group(0, b, GROUPS[3])
            nc.gpsimd.dma_start(out=out_d[0].rearrange("a k s -> k a s"),
                                in_=get_out(0)[:])

            # ---- images 1..n-1: steady state
            for i in range(1, n_img):
                for b in range(A):
                    for group in GROUPS:
                        conv1_group(i, b, group)

                # prep the next image now: its binarize + copies + prebias
                # complete during this image's conv2 window, so conv1 of the
                # next image starts without any engine-queue backlog
                if i + 1 < n_img:
                    fetch(i + 1)
                    band_acts(i + 1, 0, H)
                    xn = x_tiles[i + 1]
                    for b in range(A):
                        nc.scalar.add(out=xn[:, b, :], in_=xn[:, b, :],
                                      add=cns[b][:, 3:4])

                for b in range(A):
                    for group in GROUPS:
                        conv2_group(i, b, group)

                if i != n_img - 1:
                    nc.gpsimd.dma_start(out=out_d[i].rearrange("a k s -> k a s"),
                                        in_=get_out(i)[:])

    nc.compile()
    return nc


def _get_program(n_img):
    if n_img not in _CACHE:
        _CACHE[n_img] = _build_program(n_img)
    return _CACHE[n_img]


def _prep_consts(w1, gamma1, beta1, mean1, var1, w2, gamma2, beta2, mean2, var2):
    import ml_dtypes

    def wprep(w):
        # [O, C, 3, 3] -> [co_blk b, ci k, tap t, ci_blk i, co m], sign in fp8e4
        s = np.sign(w.astype(np.float32)).reshape(A, 128, A, 128, 9)  # [b, m, i, k, t]
        return np.ascontiguousarray(s.transpose(0, 3, 4, 2, 1)).astype(
            ml_dtypes.float8_e4m3)

    def bnfold(w, gamma, beta, mean, var):
        alpha = np.mean(np.abs(w.astype(np.float32)), axis=(1, 2, 3), dtype=np.float32)
        inv = (gamma.astype(np.float32)
               * (1.0 / np.sqrt(var.astype(np.float64) + EPS)).astype(np.float32))
        scale = alpha * inv
        bias = beta.astype(np.float32) - mean.astype(np.float32) * inv
        return scale, bias

    a1, c1 = bnfold(w1, gamma1, beta1, mean1, var1)
    a2, c2 = bnfold(w2, gamma2, beta2, mean2, var2)
    cn = np.ascontiguousarray(
        np.stack([a1, c1, a2, c2], axis=1).reshape(A, 128, 4)).astype(np.float32)
    return wprep(w1), wprep(w2), cn


def kernel(x, w1, gamma1, beta1, mean1, var1, w2, gamma2, beta2, mean2, var2):
    global LAST_RESULT
    from concourse.bass_utils import run_bass_kernel_spmd

    x, w1, gamma1, beta1, mean1, var1, w2, gamma2, beta2, mean2, var2 = (
        np.asarray(v) for v in
        (x, w1, gamma1, beta1, mean1, var1, w2, gamma2, beta2, mean2, var2))

    nc = _get_program(IMG_PER_CORE)
    w1t, w2t, cn = _prep_consts(w1, gamma1, beta1, mean1, var1,
                                w2, gamma2, beta2, mean2, var2)

    x = np.asarray(x, dtype=np.float32)
    xs = x.reshape(N_CORES, IMG_PER_CORE, A, 128, HW)
    in_maps = [
        {"x": xs[g], "w1t": w1t, "w2t": w2t, "cn": cn} for g in range(N_CORES)
    ]

    kwargs = {}
    if os.environ.get("BASS_KERNEL_TRACE"):
        _install_trace_shim()
        kwargs = dict(trace=True, tmpdir=os.environ.get("BASS_KERNEL_TRACE_DIR") or None)

    res = run_bass_kernel_spmd(nc, in_maps, list(range(N_CORES)), **kwargs)
    LAST_RESULT = res

    out = np.empty((N, C, H, W), dtype=np.float32)
    for g in range(N_CORES):
        out[g * IMG_PER_CORE:(g + 1) * IMG_PER_CORE] = (
            res.results[g]["out"].reshape(IMG_PER_CORE, C, H, W))
    return out


def _install_trace_shim():
    """This image lacks antenv.axon_hooks; recreate it so NTFF tracing works."""
    import sys, types
    if "antenv.axon_hooks" in sys.modules:
        return
    try:
        import antenv
        from trn_agent_boot.trn_boot import _ntff_profile_via_ctypes
    except ImportError:
        return
    mod = types.ModuleType("antenv.axon_hooks")
    _hook = [_ntff_profile_via_ctypes("/opt/axon/libaxon_pjrt.so")]
    mod.set_axon_ntff_profile_hook = lambda h: _hook.__setitem__(0, h)
    mod.get_axon_ntff_profile_hook = lambda: _hook[0]
    sys.modules["antenv.axon_hooks"] = mod
    antenv.axon_hooks = mod



# revision 30
# speedup vs baseline: 1.0057x; 1.0057x over previous
"""Trainium2 Bass kernel for an XNOR-Net BasicBlock (dense_cnn).

Computes, for x [64,256,56,56] (NCHW):
    h = xnor_conv3x3(x, w1) -> bn1 -> hardtanh -> xnor_conv3x3 -> bn2
    out = relu(h + x)

where xnor_conv binarizes activations with sign() and weights with
sign()*mean(|w|) (per output channel).

Strategy (v10, fp8 DoubleRow at ~157 TF/s/core; the stream of 2016
matmuls x 189ns is the roofline, so scheduling focuses on prologue,
epilogue, and keeping the PE p-state hot):
  - Data-parallel over batch: 8 images per NeuronCore x 8 cores.
  - Binarized activations (+-1) are exact in fp8e4; conv = 9 shifted
    matmuls per 3x3 tap with fp32 PSUM accumulation (exact integers).
  - perf_mode=DoubleRow contracts K=256 (both 128-channel blocks) per
    matmul: lhsT [128,2,128], rhs [128,2,448]. DoubleRow requires a 3D
    rhs AP with contiguous N, so sign planes are stored 3x, one copy per
    kj column shift, with row stride 56 (58 rows x 56 cols, borders 0).
    Window for tap (ki,kj), out-row-chunk r0 is then the contiguous run
    plane[kj][:, :, (r0+ki)*W : +N].
  - Chunks are processed in pairs sharing one 2-bank PSUM tile [128,896]
    (each matmul still targets a single bank), halving evacuation ops.
  - Epilogue fusions: conv1 evac = Sign(a1*psum + c1) on ScalarE writing
    the kj=1 plane (kj=0 copy on DVE, kj=2 on GpSimd); conv2 evac =
    DVE (psum*a2)+x' then DVE max(.,0), where x' = x + c2 is prebiased
    once per image on ScalarE. All per-channel constants (alpha, bn
    scale/bias) are folded on the host. hardtanh is a no-op for the
    final output because conv2 only consumes sign(h).
  - Prologue: N_WARM dummy matmuls hold the PE at full clock while
    image 0 streams in on a banded gpsimd DMA queue (weights ride the
    sync engine's queue); image 0 runs a hand-interleaved wave schedule
    (conv2 groups slot in once their conv1 halo exists) because all 8
    cores burst-fetch at t=0 and DMA latency dominates.
  - Steady state: image i+1's binarize/prebias are emitted between
    conv1(i) and conv2(i) so every engine queue drains before conv1 of
    the next image; the last image streams each finished conv2 group to
    DRAM from ScalarE and splits the final chunk's evacuation in half.

Layouts (per core):
  x DRAM     [8, 2, 128, 3136]   (img, c_blk, c_in_blk, h*w) fp32
  w DRAM     [2, 128, 9, 2, 128] (co_blk, ci, tap, ci_blk, co) fp8 sign
  cn DRAM    [2, 128, 4]         (co_blk, co, {a1,c1,a2,c2}) fp32
  out DRAM   [8, 2, 128, 3136]   (img, co_blk, co, h*w) fp32
"""

import os
import numpy as np

N, C, H, W = 64, 256, 56, 56
EPS = 1e-5
N_CORES = 8
IMG_PER_CORE = N // N_CORES
A = 2                     # channel blocks of 128
ROWS = H + 2              # padded rows in a plane
PLANE = ROWS * W          # 3248 (multiple of 16 for DoubleRow dim1 step)
RCH = 8                   # output rows per PSUM chunk
CHUNK = RCH * W           # 448 fp32 <= 512 (one PSUM bank)
HW = H * W
GROUPS = [(0, 1), (2, 3), (4, 5), (6,)]   # chunk pairs -> one PSUM tile
GROUPS0 = [(0,), (1,), (2, 3), (4, 5), (6,)]  # img-0 conv1: singles first
TAPS = [1, 4, 7, 0, 3, 6, 2, 5, 8]        # kj=1 taps first (plane-prep overlap)
BANDS0 = ((0, 9), (9, 24), (24, 40), (40, H))  # image-0 row bands
N_WARM = 24                               # PE p-state warm-up dummy matmuls

_CACHE = {}
LAST_RESULT = None


def _build_program(n_img):
    import concourse.bacc as bacc
    import concourse.mybir as mybir
    import concourse.tile as tile

    dt = mybir.dt
    AF = mybir.ActivationFunctionType
    OP = mybir.AluOpType
    DR = mybir.MatmulPerfMode.DoubleRow

    nc = bacc.Bacc("TRN2", target_bir_lowering=False, debug=False)

    x_d = nc.dram_tensor("x", [n_img, A, 128, HW], dt.float32, kind="ExternalInput")
    w1_d = nc.dram_tensor("w1t", [A, 128, 9, A, 128], dt.float8e4, kind="ExternalInput")
    w2_d = nc.dram_tensor("w2t", [A, 128, 9, A, 128], dt.float8e4, kind="ExternalInput")
    cn_d = nc.dram_tensor("cn", [A, 128, 4], dt.float32, kind="ExternalInput")
    out_d = nc.dram_tensor("out", [n_img, A, 128, HW], dt.float32, kind="ExternalOutput")

    with tile.TileContext(nc) as tc:
        with (
            tc.tile_pool(name="consts", bufs=1) as consts,
            tc.tile_pool(name="planes", bufs=1) as planes,
            tc.tile_pool(name="xin", bufs=2) as xin,
            tc.tile_pool(name="outp", bufs=1) as outp,
            tc.tile_pool(name="evac", bufs=3) as evac,
            tc.tile_pool(name="psum", bufs=1, space="PSUM") as psum,
        ):
            # image-0 input DMA in row bands (a-interleaved) on the gpsimd
            # queue so binarization starts as soon as the first band lands;
            # all weights/consts go down the idle sync engine's queue in
            # urgency order (w1 feeds conv1(0), w2 only from ~20us)
            x_tiles = {}
            x0 = xin.tile([128, A, HW], dt.float32, tag="x_t", name="x_0")

            ws = {}
            cns = []
            ws[(0, 0)] = consts.tile([128, 9, A, 128], dt.float8e4, tag="w0_0",
                                     name="w0_0")
            nc.sync.dma_start(out=ws[(0, 0)][:], in_=w1_d[0])
            for b in range(A):
                t = consts.tile([128, 4], dt.float32, tag=f"cn_{b}", name=f"cn_{b}")
                nc.sync.dma_start(out=t[:], in_=cn_d[b])
                cns.append(t)
            for conv, b, w_d in ((0, 1, w1_d), (1, 0, w2_d), (1, 1, w2_d)):
                t = consts.tile([128, 9, A, 128], dt.float8e4, tag=f"w{conv}_{b}",
                                name=f"w{conv}_{b}")
                nc.sync.dma_start(out=t[:], in_=w_d[b])
                ws[(conv, b)] = t

            for lo, hi in BANDS0:
                for a in range(A):
                    nc.gpsimd.dma_start(out=x0[:, a, lo * W:hi * W],
                                        in_=x_d[0, a][:, lo * W:hi * W])
            x_tiles[0] = x0

            # PE p-state warm-up: dummy matmuls on a zeroed tile keep the
            # tensor engine busy from ~7.6us so the real stream starts at
            # full clock instead of ramping through the 1.2GHz p-state
            warm = consts.tile([128, 2, CHUNK], dt.float8e4, tag="warm", name="warm")
            nc.vector.memset(warm[:], 0.0)
            wps = psum.tile([128, 512], dt.float32, tag="ps1", bufs=2, name="warm_ps")
            for _ in range(N_WARM):
                nc.tensor.matmul(wps[:, 0:CHUNK], lhsT=warm[:, :, 0:128],
                                 rhs=warm[:], start=True, stop=True,
                                 perf_mode=DR)

            # sign planes [128, kj, c_blk, 58 rows, 56 cols] fp8, borders 0,
            # ping-ponged across images. plane[kj][.., rr, j] = xpad[.., rr, j+kj]
            bxp = [planes.tile([128, 3, A, ROWS, W], dt.float8e4, tag=f"bxp{j}",
                               name=f"bxp{j}") for j in range(2)]
            s2p = [planes.tile([128, 3, A, ROWS, W], dt.float8e4, tag=f"s2p{j}",
                               name=f"s2p{j}") for j in range(2)]
            for t in (*bxp, *s2p):
                # border-only init: zero rows 0/57 (all kj) and the padding
                # columns never overwritten per image (kj0 col 0, kj2 col W-1)
                nc.vector.memset(t[:, :, :, 0, :], 0.0)
                nc.vector.memset(t[:, :, :, ROWS - 1, :], 0.0)
                nc.vector.memset(t[:, 0, :, :, 0:1], 0.0)
                nc.vector.memset(t[:, 2, :, :, W - 1:W], 0.0)

            BANK = 512

            def conv_group(src, conv, b, group, ps):
                flat = src.rearrange("p kj a r c -> p kj a (r c)")
                for n_, t_ in enumerate(TAPS):
                    ki, kj = divmod(t_, 3)
                    for gi, ch in enumerate(group):
                        r0 = ch * RCH
                        nc.tensor.matmul(
                            ps[:, gi * BANK:gi * BANK + CHUNK],
                            lhsT=ws[(conv, b)][:, t_, :, :],
                            rhs=flat[:, kj, :, (r0 + ki) * W:(r0 + ki) * W + CHUNK],
                            start=(n_ == 0), stop=(n_ == 8),
                            perf_mode=DR,
                        )

            def psum_tile(group, nm):
                # chunks live at bank-aligned offsets; tail 64 fp32/bank unused
                return psum.tile([128, len(group) * BANK], dt.float32,
                                 tag=f"ps{len(group)}", bufs=3 if len(group) > 1 else 2,
                                 name=nm)

            def psum_chunks(ps, group):
                # [128, G, 448] view of the used part of each bank
                return ps.rearrange("p (g x) -> p g x", x=BANK)[:, :, 0:CHUNK]

            def fetch(i):
                if i not in x_tiles:
                    x_t = xin.tile([128, A, HW], dt.float32, tag="x_t", name=f"x_{i}")
                    nc.gpsimd.dma_start(out=x_t[:],
                                        in_=x_d[i].rearrange("a k s -> k a s"))
                    x_tiles[i] = x_t
                return x_tiles[i]

            def band_acts(i, lo, hi):
                # binarize rows [lo,hi): kj=1 sign on ScalarE, kj=0/2 as DVE
                # shifted copies, split per c_blk so the a=0 copies overlap
                # the a=1 sign pass
                j = i % 2
                xv = x_tiles[i].rearrange("p a (r c) -> p a r c", c=W)
                for a in range(A):
                    nc.scalar.activation(
                        out=bxp[j][:, 1, a, 1 + lo:1 + hi, :],
                        in_=xv[:, a, lo:hi, :],
                        func=AF.Sign,
                    )
                for a in range(A):
                    nc.vector.tensor_copy(
                        out=bxp[j][:, 0, a, 1 + lo:1 + hi, 1:W],
                        in_=bxp[j][:, 1, a, 1 + lo:1 + hi, 0:W - 1])
                    nc.vector.tensor_copy(
                        out=bxp[j][:, 2, a, 1 + lo:1 + hi, 0:W - 1],
                        in_=bxp[j][:, 1, a, 1 + lo:1 + hi, 1:W])

            def conv1_group(i, b, group):
                # conv1 -> fused bn1+sign -> s2p (x3 shifted)
                j = i % 2
                gr = len(group) * RCH
                r0 = group[0] * RCH
                ps = psum_tile(group, f"ps1_{i}_{b}_{group[0]}")
                conv_group(bxp[j], 0, b, group, ps)
                nc.scalar.activation(
                    out=s2p[j][:, 1, b, 1 + r0:1 + r0 + gr, :],
                    in_=psum_chunks(ps, group).rearrange(
                        "p g (r c) -> p g r c", c=W),
                    func=AF.Sign,
                    bias=cns[b][:, 1:2],
                    scale=cns[b][:, 0:1],
                )
                nc.vector.tensor_copy(
                    out=s2p[j][:, 0, b, 1 + r0:1 + r0 + gr, 1:W],
                    in_=s2p[j][:, 1, b, 1 + r0:1 + r0 + gr, 0:W - 1])
                nc.vector.tensor_copy(
                    out=s2p[j][:, 2, b, 1 + r0:1 + r0 + gr, 0:W - 1],
                    in_=s2p[j][:, 1, b, 1 + r0:1 + r0 + gr, 1:W])

            out_tiles = {}

            def get_out(i):
                if i not in out_tiles:
                    out_tiles[i] = outp.tile([128, A, HW], dt.float32,
                                             tag="out_t", name=f"out_{i}")
                return out_tiles[i]

            def conv2_group(i, b, group):
                # conv2 -> DVE (psum*a2)+x' -> DVE max(.,0); the last image
                # streams each finished group to DRAM (ScalarE-issued), and
                # its very last chunk evacuates in two halves for a tighter
                # tail after the final matmul
                j = i % 2
                last = i == n_img - 1
                out_t = get_out(i)
                x_t = x_tiles[i]
                gn = len(group) * CHUNK
                s0 = group[0] * CHUNK
                ps = psum_tile(group, f"ps2_{i}_{b}_{group[0]}")
                conv_group(s2p[j], 1, b, group, ps)
                rr = evac.tile([128, 2 * CHUNK], dt.float32, tag="rr",
                               name=f"rr_{i}_{b}_{group[0]}")
                final = last and b == A - 1 and group is GROUPS[-1]
                pv = psum_chunks(ps, group)
                for pp in range(2 if final else 1):
                    on = gn // 2 if final else gn
                    ro = pp * on
                    o0 = s0 + ro
                    nc.vector.scalar_tensor_tensor(
                        out=rr[:, ro:ro + on],
                        in0=pv[:, :, ro:ro + on] if final else pv,
                        scalar=cns[b][:, 2:3],
                        in1=x_t[:, b, o0:o0 + on],
                        op0=OP.mult, op1=OP.add)
                    nc.vector.tensor_scalar_max(
                        out=out_t[:, b, o0:o0 + on],
                        in0=rr[:, ro:ro + on],
                        scalar1=0.0)
                    if last:
                        nc.scalar.dma_start(
                            out=out_d[i, b][:, o0:o0 + on],
                            in_=out_t[:, b, o0:o0 + on])

            # ---- image 0: hand-interleaved wave schedule. The prologue is
            # DMA-latency-bound (all 8 cores burst-fetch at once), so conv2
            # groups interleave with conv1 as soon as their halo (conv1
            # chunks 2k-1..2k+2) exists, giving the PE extra work per input
            # byte while the bands stream in. ScalarE order: bands early,
            # evacs/prebias in the gaps.
            fetch(0)
            band_acts(0, *BANDS0[0])
            band_acts(0, *BANDS0[1])
            for b in range(A):
                conv1_group(0, b, GROUPS0[0])
            band_acts(0, *BANDS0[2])
            for b in range(A):
                conv1_group(0, b, GROUPS0[1])
            band_acts(0, *BANDS0[3])
            for b in range(A):
                conv1_group(0, b, GROUPS0[2])
            # prebias (x' = x + c2) MUST precede the first conv2 group's
            # evacuation in program order (its stt reads x'); ScalarE runs
            # it in the gap while the PE does conv2(0,1) matmuls
            for b in range(A):
                nc.scalar.add(out=x0[:, b, :], in_=x0[:, b, :],
                              add=cns[b][:, 3:4])
            for b in range(A):
                conv2_group(0, b, GROUPS[0])
            for b in range(A):
                conv1_group(0, b, GROUPS0[3])
            for b in range(A):
                conv1_group(0, b, GROUPS0[4])
            for b in range(A):
                conv2_group(0, b, GROUPS[1])
            for b in range(A):
                conv2_group(0, b, GROUPS[2])

            # prep image 1 while image 0's tail computes
            fetch(1)
            band_acts(1, 0, H)
            for b in range(A):
                nc.scalar.add(out=x_tiles[1][:, b, :], in_=x_tiles[1][:, b, :],
                              add=cns[b][:, 3:4])
            for b in range(A):
                conv2_group(0, b, GROUPS[3])
            nc.gpsimd.dma_start(out=out_d[0].rearrange("a k s -> k a s"),
                                in_=get_out(0)[:])

            # ---- images 1..n-1: steady state
            for i in range(1, n_img):
                for b in range(A):
                    for group in GROUPS:
                        conv1_group(i, b, group)

                # prep the next image now: its binarize + copies + prebias
                # complete during this image's conv2 window, so conv1 of the
                # next image starts without any engine-queue backlog
                if i + 1 < n_img:
                    fetch(i + 1)
                    band_acts(i + 1, 0, H)
                    xn = x_tiles[i + 1]
                    for b in range(A):
                        nc.scalar.add(out=xn[:, b, :], in_=xn[:, b, :],
                                      add=cns[b][:, 3:4])

                for b in range(A):
                    for group in GROUPS:
                        conv2_group(i, b, group)

                if i != n_img - 1:
                    nc.gpsimd.dma_start(out=out_d[i].rearrange("a k s -> k a s"),
                                        in_=get_out(i)[:])

    nc.compile()
    return nc


def _get_program(n_img):
    if n_img not in _CACHE:
        _CACHE[n_img] = _build_program(n_img)
    return _CACHE[n_img]


def _prep_consts(w1, gamma1, beta1, mean1, var1, w2, gamma2, beta2, mean2, var2):
    import ml_dtypes

    def wprep(w):
        # [O, C, 3, 3] -> [co_blk b, ci k, tap t, ci_blk i, co m], sign in fp8e4
        s = np.sign(w.astype(np.float32)).reshape(A, 128, A, 128, 9)  # [b, m, i, k, t]
        return np.ascontiguousarray(s.transpose(0, 3, 4, 2, 1)).astype(
            ml_dtypes.float8_e4m3)

    def bnfold(w, gamma, beta, mean, var):
        alpha = np.mean(np.abs(w.astype(np.float32)), axis=(1, 2, 3), dtype=np.float32)
        inv = (gamma.astype(np.float32)
               * (1.0 / np.sqrt(var.astype(np.float64) + EPS)).astype(np.float32))
        scale = alpha * inv
        bias = beta.astype(np.float32) - mean.astype(np.float32) * inv
        return scale, bias

    a1, c1 = bnfold(w1, gamma1, beta1, mean1, var1)
    a2, c2 = bnfold(w2, gamma2, beta2, mean2, var2)
    cn = np.ascontiguousarray(
        np.stack([a1, c1, a2, c2], axis=1).reshape(A, 128, 4)).astype(np.float32)
    return wprep(w1), wprep(w2), cn


def kernel(x, w1, gamma1, beta1, mean1, var1, w2, gamma2, beta2, mean2, var2):
    global LAST_RESULT
    from concourse.bass_utils import run_bass_kernel_spmd

    x, w1, gamma1, beta1, mean1, var1, w2, gamma2, beta2, mean2, var2 = (
        np.asarray(v) for v in
        (x, w1, gamma1, beta1, mean1, var1, w2, gamma2, beta2, mean2, var2))

    nc = _get_program(IMG_PER_CORE)
    w1t, w2t, cn = _prep_consts(w1, gamma1, beta1, mean1, var1,
                                w2, gamma2, beta2, mean2, var2)

    x = np.asarray(x, dtype=np.float32)
    xs = x.reshape(N_CORES, IMG_PER_CORE, A, 128, HW)
    in_maps = [
        {"x": xs[g], "w1t": w1t, "w2t": w2t, "cn": cn} for g in range(N_CORES)
    ]

    kwargs = {}
    if os.environ.get("BASS_KERNEL_TRACE"):
        _install_trace_shim()
        kwargs = dict(trace=True, tmpdir=os.environ.get("BASS_KERNEL_TRACE_DIR") or None)

    res = run_bass_kernel_spmd(nc, in_maps, list(range(N_CORES)), **kwargs)
    LAST_RESULT = res

    out = np.empty((N, C, H, W), dtype=np.float32)
    for g in range(N_CORES):
        out[g * IMG_PER_CORE:(g + 1) * IMG_PER_CORE] = (
            res.results[g]["out"].reshape(IMG_PER_CORE, C, H, W))
    return out


def _install_trace_shim():
    """This image lacks antenv.axon_hooks; recreate it so NTFF tracing works."""
    import sys, types
    if "antenv.axon_hooks" in sys.modules:
        return
    try:
        import antenv
        from trn_agent_boot.trn_boot import _ntff_profile_via_ctypes
    except ImportError:
        return
    mod = types.ModuleType("antenv.axon_hooks")
    _hook = [_ntff_profile_via_ctypes("/opt/axon/libaxon_pjrt.so")]
    mod.set_axon_ntff_profile_hook = lambda h: _hook.__setitem__(0, h)
    mod.get_axon_ntff_profile_hook = lambda: _hook[0]
    sys.modules["antenv.axon_hooks"] = mod
    antenv.axon_hooks = mod



# revision 31
# speedup vs baseline: 1.0106x; 1.0048x over previous
"""Trainium2 Bass kernel for an XNOR-Net BasicBlock (dense_cnn).

Computes, for x [64,256,56,56] (NCHW):
    h = xnor_conv3x3(x, w1) -> bn1 -> hardtanh -> xnor_conv3x3 -> bn2
    out = relu(h + x)

where xnor_conv binarizes activations with sign() and weights with
sign()*mean(|w|) (per output channel).

Strategy (v10, fp8 DoubleRow at ~157 TF/s/core; the stream of 2016
matmuls x 189ns is the roofline, so scheduling focuses on prologue,
epilogue, and keeping the PE p-state hot):
  - Data-parallel over batch: 8 images per NeuronCore x 8 cores.
  - Binarized activations (+-1) are exact in fp8e4; conv = 9 shifted
    matmuls per 3x3 tap with fp32 PSUM accumulation (exact integers).
  - perf_mode=DoubleRow contracts K=256 (both 128-channel blocks) per
    matmul: lhsT [128,2,128], rhs [128,2,448]. DoubleRow requires a 3D
    rhs AP with contiguous N, so sign planes are stored 3x, one copy per
    kj column shift, with row stride 56 (58 rows x 56 cols, borders 0).
    Window for tap (ki,kj), out-row-chunk r0 is then the contiguous run
    plane[kj][:, :, (r0+ki)*W : +N].
  - Chunks are processed in pairs sharing one 2-bank PSUM tile [128,896]
    (each matmul still targets a single bank), halving evacuation ops.
  - Epilogue fusions: conv1 evac = Sign(a1*psum + c1) on ScalarE writing
    the kj=1 plane (kj=0 copy on DVE, kj=2 on GpSimd); conv2 evac =
    DVE (psum*a2)+x' then DVE max(.,0), where x' = x + c2 is prebiased
    once per image on ScalarE. All per-channel constants (alpha, bn
    scale/bias) are folded on the host. hardtanh is a no-op for the
    final output because conv2 only consumes sign(h).
  - Prologue: N_WARM dummy matmuls hold the PE at full clock while
    image 0 streams in on a banded gpsimd DMA queue (weights ride the
    sync engine's queue); image 0 runs a hand-interleaved wave schedule
    (conv2 groups slot in once their conv1 halo exists) because all 8
    cores burst-fetch at t=0 and DMA latency dominates.
  - Steady state: image i+1's binarize/prebias are emitted between
    conv1(i) and conv2(i) so every engine queue drains before conv1 of
    the next image; the last image streams each finished conv2 group to
    DRAM from ScalarE and splits the final chunk's evacuation in half.

Layouts (per core):
  x DRAM     [8, 2, 128, 3136]   (img, c_blk, c_in_blk, h*w) fp32
  w DRAM     [2, 128, 9, 2, 128] (co_blk, ci, tap, ci_blk, co) fp8 sign
  cn DRAM    [2, 128, 4]         (co_blk, co, {a1,c1,a2,c2}) fp32
  out DRAM   [8, 2, 128, 3136]   (img, co_blk, co, h*w) fp32
"""

import os
import numpy as np

N, C, H, W = 64, 256, 56, 56
EPS = 1e-5
N_CORES = 8
IMG_PER_CORE = N // N_CORES
A = 2                     # channel blocks of 128
ROWS = H + 2              # padded rows in a plane
PLANE = ROWS * W          # 3248 (multiple of 16 for DoubleRow dim1 step)
RCH = 8                   # output rows per PSUM chunk
CHUNK = RCH * W           # 448 fp32 <= 512 (one PSUM bank)
HW = H * W
GROUPS = [(0, 1), (2, 3), (4, 5), (6,)]   # chunk pairs -> one PSUM tile
GROUPS0 = [(0,), (1,), (2, 3), (4, 5), (6,)]  # img-0 conv1: singles first
TAPS = [1, 4, 7, 0, 3, 6, 2, 5, 8]        # kj=1 taps first (plane-prep overlap)
BANDS0 = ((0, 9), (9, 24), (24, 40), (40, H))  # image-0 row bands
N_WARM = 24                               # PE p-state warm-up dummy matmuls

_CACHE = {}
LAST_RESULT = None


def _build_program(n_img):
    import concourse.bacc as bacc
    import concourse.mybir as mybir
    import concourse.tile as tile

    dt = mybir.dt
    AF = mybir.ActivationFunctionType
    OP = mybir.AluOpType
    DR = mybir.MatmulPerfMode.DoubleRow

    nc = bacc.Bacc("TRN2", target_bir_lowering=False, debug=False)

    x_d = nc.dram_tensor("x", [n_img, A, 128, HW], dt.float32, kind="ExternalInput")
    w1_d = nc.dram_tensor("w1t", [A, 128, 9, A, 128], dt.float8e4, kind="ExternalInput")
    w2_d = nc.dram_tensor("w2t", [A, 128, 9, A, 128], dt.float8e4, kind="ExternalInput")
    cn_d = nc.dram_tensor("cn", [A, 128, 4], dt.float32, kind="ExternalInput")
    out_d = nc.dram_tensor("out", [n_img, A, 128, HW], dt.float32, kind="ExternalOutput")

    with tile.TileContext(nc) as tc:
        with (
            tc.tile_pool(name="consts", bufs=1) as consts,
            tc.tile_pool(name="planes", bufs=1) as planes,
            tc.tile_pool(name="xin", bufs=2) as xin,
            tc.tile_pool(name="outp", bufs=1) as outp,
            tc.tile_pool(name="evac", bufs=3) as evac,
            tc.tile_pool(name="psum", bufs=1, space="PSUM") as psum,
        ):
            # image-0 input DMA in row bands (a-interleaved) on the gpsimd
            # queue so binarization starts as soon as the first band lands;
            # all weights/consts go down the idle sync engine's queue in
            # urgency order (w1 feeds conv1(0), w2 only from ~20us)
            x_tiles = {}
            x0 = xin.tile([128, A, HW], dt.float32, tag="x_t", name="x_0")

            ws = {}
            cns = []
            ws[(0, 0)] = consts.tile([128, 9, A, 128], dt.float8e4, tag="w0_0",
                                     name="w0_0")
            nc.sync.dma_start(out=ws[(0, 0)][:], in_=w1_d[0])
            for b in range(A):
                t = consts.tile([128, 4], dt.float32, tag=f"cn_{b}", name=f"cn_{b}")
                nc.sync.dma_start(out=t[:], in_=cn_d[b])
                cns.append(t)
            for conv, b, w_d in ((0, 1, w1_d), (1, 0, w2_d), (1, 1, w2_d)):
                t = consts.tile([128, 9, A, 128], dt.float8e4, tag=f"w{conv}_{b}",
                                name=f"w{conv}_{b}")
                nc.sync.dma_start(out=t[:], in_=w_d[b])
                ws[(conv, b)] = t

            for lo, hi in BANDS0:
                for a in range(A):
                    nc.gpsimd.dma_start(out=x0[:, a, lo * W:hi * W],
                                        in_=x_d[0, a][:, lo * W:hi * W])
            x_tiles[0] = x0

            # PE p-state warm-up: dummy matmuls on a zeroed tile keep the
            # tensor engine busy from ~7.6us so the real stream starts at
            # full clock instead of ramping through the 1.2GHz p-state
            warm = consts.tile([128, 2, CHUNK], dt.float8e4, tag="warm", name="warm")
            nc.vector.memset(warm[:], 0.0)
            wps = psum.tile([128, 512], dt.float32, tag="ps1", bufs=2, name="warm_ps")
            for _ in range(N_WARM):
                nc.tensor.matmul(wps[:, 0:CHUNK], lhsT=warm[:, :, 0:128],
                                 rhs=warm[:], start=True, stop=True,
                                 perf_mode=DR)

            # sign planes [128, kj, c_blk, 58 rows, 56 cols] fp8, borders 0,
            # ping-ponged across images. plane[kj][.., rr, j] = xpad[.., rr, j+kj]
            bxp = [planes.tile([128, 3, A, ROWS, W], dt.float8e4, tag=f"bxp{j}",
                               name=f"bxp{j}") for j in range(2)]
            s2p = [planes.tile([128, 3, A, ROWS, W], dt.float8e4, tag=f"s2p{j}",
                               name=f"s2p{j}") for j in range(2)]
            for t in (*bxp, *s2p):
                # border-only init: zero rows 0/57 (all kj) and the padding
                # columns never overwritten per image (kj0 col 0, kj2 col W-1)
                nc.vector.memset(t[:, :, :, 0, :], 0.0)
                nc.vector.memset(t[:, :, :, ROWS - 1, :], 0.0)
                nc.vector.memset(t[:, 0, :, :, 0:1], 0.0)
                nc.vector.memset(t[:, 2, :, :, W - 1:W], 0.0)

            BANK = 512

            def conv_group(src, conv, b, group, ps):
                flat = src.rearrange("p kj a r c -> p kj a (r c)")
                for n_, t_ in enumerate(TAPS):
                    ki, kj = divmod(t_, 3)
                    for gi, ch in enumerate(group):
                        r0 = ch * RCH
                        nc.tensor.matmul(
                            ps[:, gi * BANK:gi * BANK + CHUNK],
                            lhsT=ws[(conv, b)][:, t_, :, :],
                            rhs=flat[:, kj, :, (r0 + ki) * W:(r0 + ki) * W + CHUNK],
                            start=(n_ == 0), stop=(n_ == 8),
                            perf_mode=DR,
                        )

            def psum_tile(group, nm):
                # chunks live at bank-aligned offsets; tail 64 fp32/bank unused
                return psum.tile([128, len(group) * BANK], dt.float32,
                                 tag=f"ps{len(group)}", bufs=3 if len(group) > 1 else 2,
                                 name=nm)

            def psum_chunks(ps, group):
                # [128, G, 448] view of the used part of each bank
                return ps.rearrange("p (g x) -> p g x", x=BANK)[:, :, 0:CHUNK]

            def fetch(i):
                if i not in x_tiles:
                    x_t = xin.tile([128, A, HW], dt.float32, tag="x_t", name=f"x_{i}")
                    nc.gpsimd.dma_start(out=x_t[:],
                                        in_=x_d[i].rearrange("a k s -> k a s"))
                    x_tiles[i] = x_t
                return x_tiles[i]

            def band_acts(i, lo, hi):
                # binarize rows [lo,hi): kj=1 sign on ScalarE, kj=0/2 as DVE
                # shifted copies, split per c_blk so the a=0 copies overlap
                # the a=1 sign pass
                j = i % 2
                xv = x_tiles[i].rearrange("p a (r c) -> p a r c", c=W)
                for a in range(A):
                    nc.scalar.activation(
                        out=bxp[j][:, 1, a, 1 + lo:1 + hi, :],
                        in_=xv[:, a, lo:hi, :],
                        func=AF.Sign,
                    )
                for a in range(A):
                    nc.vector.tensor_copy(
                        out=bxp[j][:, 0, a, 1 + lo:1 + hi, 1:W],
                        in_=bxp[j][:, 1, a, 1 + lo:1 + hi, 0:W - 1])
                    nc.vector.tensor_copy(
                        out=bxp[j][:, 2, a, 1 + lo:1 + hi, 0:W - 1],
                        in_=bxp[j][:, 1, a, 1 + lo:1 + hi, 1:W])

            def conv1_group(i, b, group):
                # conv1 -> fused bn1+sign -> s2p (x3 shifted)
                j = i % 2
                gr = len(group) * RCH
                r0 = group[0] * RCH
                ps = psum_tile(group, f"ps1_{i}_{b}_{group[0]}")
                conv_group(bxp[j], 0, b, group, ps)
                nc.scalar.activation(
                    out=s2p[j][:, 1, b, 1 + r0:1 + r0 + gr, :],
                    in_=psum_chunks(ps, group).rearrange(
                        "p g (r c) -> p g r c", c=W),
                    func=AF.Sign,
                    bias=cns[b][:, 1:2],
                    scale=cns[b][:, 0:1],
                )
                nc.vector.tensor_copy(
                    out=s2p[j][:, 0, b, 1 + r0:1 + r0 + gr, 1:W],
                    in_=s2p[j][:, 1, b, 1 + r0:1 + r0 + gr, 0:W - 1])
                nc.vector.tensor_copy(
                    out=s2p[j][:, 2, b, 1 + r0:1 + r0 + gr, 0:W - 1],
                    in_=s2p[j][:, 1, b, 1 + r0:1 + r0 + gr, 1:W])

            out_tiles = {}

            def get_out(i):
                if i not in out_tiles:
                    out_tiles[i] = outp.tile([128, A, HW], dt.float32,
                                             tag="out_t", name=f"out_{i}")
                return out_tiles[i]

            def conv2_group(i, b, group):
                # conv2 -> DVE (psum*a2)+x' -> DVE max(.,0); the last image
                # streams each finished group to DRAM (ScalarE-issued), and
                # its very last chunk evacuates in two halves for a tighter
                # tail after the final matmul
                j = i % 2
                last = i == n_img - 1
                out_t = get_out(i)
                x_t = x_tiles[i]
                gn = len(group) * CHUNK
                s0 = group[0] * CHUNK
                ps = psum_tile(group, f"ps2_{i}_{b}_{group[0]}")
                conv_group(s2p[j], 1, b, group, ps)
                rr = evac.tile([128, 2 * CHUNK], dt.float32, tag="rr",
                               name=f"rr_{i}_{b}_{group[0]}")
                final = last and b == A - 1 and group is GROUPS[-1]
                pv = psum_chunks(ps, group)
                for pp in range(2 if final else 1):
                    on = gn // 2 if final else gn
                    ro = pp * on
                    o0 = s0 + ro
                    nc.vector.scalar_tensor_tensor(
                        out=rr[:, ro:ro + on],
                        in0=pv[:, :, ro:ro + on] if final else pv,
                        scalar=cns[b][:, 2:3],
                        in1=x_t[:, b, o0:o0 + on],
                        op0=OP.mult, op1=OP.add)
                    nc.vector.tensor_scalar_max(
                        out=out_t[:, b, o0:o0 + on],
                        in0=rr[:, ro:ro + on],
                        scalar1=0.0)
                    if last:
                        nc.scalar.dma_start(
                            out=out_d[i, b][:, o0:o0 + on],
                            in_=out_t[:, b, o0:o0 + on])

            # ---- image 0: hand-interleaved wave schedule. The prologue is
            # DMA-latency-bound (all 8 cores burst-fetch at once), so conv2
            # groups interleave with conv1 as soon as their halo (conv1
            # chunks 2k-1..2k+2) exists, giving the PE extra work per input
            # byte while the bands stream in. ScalarE order: bands early,
            # evacs/prebias in the gaps.
            fetch(0)
            band_acts(0, *BANDS0[0])
            band_acts(0, *BANDS0[1])
            for b in range(A):
                conv1_group(0, b, GROUPS0[0])
            band_acts(0, *BANDS0[2])
            for b in range(A):
                conv1_group(0, b, GROUPS0[1])
            band_acts(0, *BANDS0[3])
            for b in range(A):
                conv1_group(0, b, GROUPS0[2])
            # prebias (x' = x + c2) MUST precede the first conv2 group's
            # evacuation in program order (its stt reads x'); ScalarE runs
            # it in the gap while the PE does conv2(0,1) matmuls
            for b in range(A):
                nc.scalar.add(out=x0[:, b, :], in_=x0[:, b, :],
                              add=cns[b][:, 3:4])
            for b in range(A):
                conv2_group(0, b, GROUPS[0])
            for b in range(A):
                conv1_group(0, b, GROUPS0[3])
            for b in range(A):
                conv1_group(0, b, GROUPS0[4])
            for b in range(A):
                conv2_group(0, b, GROUPS[1])

            # prep image 1 here: its DVE plane copies drain before image 0's
            # last conv2 evacuations, so conv1(1) starts without a stall
            fetch(1)
            band_acts(1, 0, H)
            for b in range(A):
                nc.scalar.add(out=x_tiles[1][:, b, :], in_=x_tiles[1][:, b, :],
                              add=cns[b][:, 3:4])

            for b in range(A):
                conv2_group(0, b, GROUPS[2])
            for b in range(A):
                conv2_group(0, b, GROUPS[3])
            nc.gpsimd.dma_start(out=out_d[0].rearrange("a k s -> k a s"),
                                in_=get_out(0)[:])

            # ---- images 1..n-1: steady state
            for i in range(1, n_img):
                for b in range(A):
                    for group in GROUPS:
                        conv1_group(i, b, group)

                # prep the next image now: its binarize + copies + prebias
                # complete during this image's conv2 window, so conv1 of the
                # next image starts without any engine-queue backlog
                if i + 1 < n_img:
                    fetch(i + 1)
                    band_acts(i + 1, 0, H)
                    xn = x_tiles[i + 1]
                    for b in range(A):
                        nc.scalar.add(out=xn[:, b, :], in_=xn[:, b, :],
                                      add=cns[b][:, 3:4])

                for b in range(A):
                    for group in GROUPS:
                        conv2_group(i, b, group)

                if i != n_img - 1:
                    nc.gpsimd.dma_start(out=out_d[i].rearrange("a k s -> k a s"),
                                        in_=get_out(i)[:])

    nc.compile()
    return nc


def _get_program(n_img):
    if n_img not in _CACHE:
        _CACHE[n_img] = _build_program(n_img)
    return _CACHE[n_img]


def _prep_consts(w1, gamma1, beta1, mean1, var1, w2, gamma2, beta2, mean2, var2):
    import ml_dtypes

    def wprep(w):
        # [O, C, 3, 3] -> [co_blk b, ci k, tap t, ci_blk i, co m], sign in fp8e4
        s = np.sign(w.astype(np.float32)).reshape(A, 128, A, 128, 9)  # [b, m, i, k, t]
        return np.ascontiguousarray(s.transpose(0, 3, 4, 2, 1)).astype(
            ml_dtypes.float8_e4m3)

    def bnfold(w, gamma, beta, mean, var):
        alpha = np.mean(np.abs(w.astype(np.float32)), axis=(1, 2, 3), dtype=np.float32)
        inv = (gamma.astype(np.float32)
               * (1.0 / np.sqrt(var.astype(np.float64) + EPS)).astype(np.float32))
        scale = alpha * inv
        bias = beta.astype(np.float32) - mean.astype(np.float32) * inv
        return scale, bias

    a1, c1 = bnfold(w1, gamma1, beta1, mean1, var1)
    a2, c2 = bnfold(w2, gamma2, beta2, mean2, var2)
    cn = np.ascontiguousarray(
        np.stack([a1, c1, a2, c2], axis=1).reshape(A, 128, 4)).astype(np.float32)
    return wprep(w1), wprep(w2), cn


def kernel(x, w1, gamma1, beta1, mean1, var1, w2, gamma2, beta2, mean2, var2):
    global LAST_RESULT
    from concourse.bass_utils import run_bass_kernel_spmd

    x, w1, gamma1, beta1, mean1, var1, w2, gamma2, beta2, mean2, var2 = (
        np.asarray(v) for v in
        (x, w1, gamma1, beta1, mean1, var1, w2, gamma2, beta2, mean2, var2))

    nc = _get_program(IMG_PER_CORE)
    w1t, w2t, cn = _prep_consts(w1, gamma1, beta1, mean1, var1,
                                w2, gamma2, beta2, mean2, var2)

    x = np.asarray(x, dtype=np.float32)
    xs = x.reshape(N_CORES, IMG_PER_CORE, A, 128, HW)
    in_maps = [
        {"x": xs[g], "w1t": w1t, "w2t": w2t, "cn": cn} for g in range(N_CORES)
    ]

    kwargs = {}
    if os.environ.get("BASS_KERNEL_TRACE"):
        _install_trace_shim()
        kwargs = dict(trace=True, tmpdir=os.environ.get("BASS_KERNEL_TRACE_DIR") or None)

    res = run_bass_kernel_spmd(nc, in_maps, list(range(N_CORES)), **kwargs)
    LAST_RESULT = res

    out = np.empty((N, C, H, W), dtype=np.float32)
    for g in range(N_CORES):
        out[g * IMG_PER_CORE:(g + 1) * IMG_PER_CORE] = (
            res.results[g]["out"].reshape(IMG_PER_CORE, C, H, W))
    return out


def _install_trace_shim():
    """This image lacks antenv.axon_hooks; recreate it so NTFF tracing works."""
    import sys, types
    if "antenv.axon_hooks" in sys.modules:
        return
    try:
        import antenv
        from trn_agent_boot.trn_boot import _ntff_profile_via_ctypes
    except ImportError:
        return
    mod = types.ModuleType("antenv.axon_hooks")
    _hook = [_ntff_profile_via_ctypes("/opt/axon/libaxon_pjrt.so")]
    mod.set_axon_ntff_profile_hook = lambda h: _hook.__setitem__(0, h)
    mod.get_axon_ntff_profile_hook = lambda: _hook[0]
    sys.modules["antenv.axon_hooks"] = mod
    antenv.axon_hooks = mod

